# revision 24
# baseline (speedup 1.0000x reference)
"""Trainium2 Bass kernel for nn_DeepEdgeCongestionGNN (6-layer GCN + edge MLP).

Strategy (8 NeuronCores, SPMD):
  - Nodes sharded by graph (2048 graphs = 61440 nodes per core), natural
    order. Per layer the full fp16 node table (chunk-interleaved rows) is
    AllGather-replicated; each core gathers the rows its edges need with
    dma_gather (int16 in-window indices; the 491520-row table splits into
    15 windows of 32768 rows).
  - Entries are sorted by (window, dst 512-node block) and padded to
    128-entry groups that are window-pure and block-pure. One dma_gather
    per (window, 8-block range) fetches ~1024 rows; a 512-wide one-hot
    matmul scatters each group into the block's PSUM bank (feat x node),
    self-loops enter via identity matmuls on the residual tile.
  - y^T = W^T s^T, fused BN+ReLU on ACT, transpose back node-major with
    the residual added on the PE, store shard, AllGather chunks.
  - Final edge MLP gathers h6[u],h6[v] via dma_gather (edges sorted by
    (u-window, v-window) class); host un-permutes the padded output.
  - The bass program is built per input (structure depends on edge counts)
    but is identical on all cores: group counts take the max over cores.
"""
import sys
import types

import numpy as np

sys.path.insert(0, "/opt/trn_rl_repo")

# --- shim antenv.axon_hooks (absent in this image) so trace=True works ---
import antenv
if "antenv.axon_hooks" not in sys.modules:
    _hookmod = types.ModuleType("antenv.axon_hooks")
    _hookmod._hook = None
    def _set(h): _hookmod._hook = h
    def _get(): return _hookmod._hook
    _hookmod.set_axon_ntff_profile_hook = _set
    _hookmod.get_axon_ntff_profile_hook = _get
    sys.modules["antenv.axon_hooks"] = _hookmod
    antenv.axon_hooks = _hookmod
    try:
        from trn_agent_boot.trn_boot import _ntff_profile_via_ctypes
        _hookmod._hook = _ntff_profile_via_ctypes("/opt/axon/libaxon_pjrt.so")
    except Exception:
        pass

import concourse.bass as bass
import concourse.bacc as bacc
import concourse.mybir as mybir
import concourse.tile as tile
from concourse.bass_utils import run_bass_kernel_spmd

F16 = mybir.dt.float16
F32 = mybir.dt.float32
I32 = mybir.dt.int32
I16 = mybir.dt.int16

NCORES = 8
NPG = 30                    # nodes per graph
G = 16384                   # graphs
N = G * NPG                 # 491520 nodes
GPC = G // NCORES           # graphs per core
NSH = GPC * NPG             # 61440 nodes per core
NBLK = NSH // 512           # 120 psum blocks of 512 nodes
LAYERS = 6
HID = 128
SPLIT = 4                   # AllGather chunks per layer
CHSH = NSH // SPLIT         # shard rows per AG chunk
BPC = NBLK // SPLIT         # blocks per AG chunk
W0 = 32768                  # dma_gather window rows (int16 idx range)
NWIN = N // W0              # 15 table windows
BRSZ = 8                    # blocks per gather range
NBR = NBLK // BRSZ          # 15 ranges
FW0 = 32768                 # final-stage window over NSH rows
NFW = 2                     # final windows (61440 = 32768 + 28672)
NOUT = GPC * 41             # 83968 output rows per core
BN_EPS = 1e-5

BRANCH_U = np.array([0,0,1,2,1,1,3,5,5,6,6,6,6,8,8,9,11,11,11,11,13,15,14,17,
                     18,9,9,21,14,21,22,23,24,24,27,26,26,28,26,7,5],
                    dtype=np.int64)
BRANCH_V = np.array([1,2,3,3,4,5,5,6,7,7,8,9,27,9,10,10,12,13,15,16,14,16,17,
                     18,19,19,20,20,22,21,23,23,24,26,26,29,28,29,27,27,8],
                    dtype=np.int64)

_CACHE = {}


def _table_row(g):
    """Global node id -> row in the chunk-interleaved AllGather table."""
    k = g // NSH
    l = g % NSH
    return (l // CHSH) * (CHSH * NCORES) + k * CHSH + (l % CHSH)


def _prep(x, edge_index):
    src = np.ascontiguousarray(edge_index[0]).astype(np.int64)
    dst = np.ascontiguousarray(edge_index[1]).astype(np.int64)

    indeg = np.bincount(dst, minlength=N).astype(np.int64)
    deg = (indeg + 1).astype(np.float64)
    dinv = (1.0 / np.sqrt(deg)).astype(np.float32)

    e_k = dst // NSH                       # consumer core
    e_l = dst % NSH
    e_blk = (e_l // 512).astype(np.int64)
    e_dloc = (e_l % 512).astype(np.int64)
    srow = _table_row(src)
    e_w = srow // W0
    e_r16 = (srow % W0).astype(np.int64)
    e_norm = (dinv[src] * dinv[dst]).astype(np.float32)

    # counts per (core, window, block)
    key_wb = e_w * NBLK + e_blk
    cnt = np.zeros((NCORES, NWIN * NBLK), np.int64)
    for k in range(NCORES):
        m = e_k == k
        cnt[k] = np.bincount(key_wb[m], minlength=NWIN * NBLK)
    g_wb = np.ceil(cnt.max(axis=0) / 128).astype(np.int64)  # shared structure
    g_wb = g_wb.reshape(NWIN, NBLK)

    # column enumeration: range-major, then window, then block, then group
    col_of = {}
    blockcols = [[] for _ in range(NBLK)]   # per block: [(col, w)...]
    calls = [[] for _ in range(NBR)]        # per range: [(w, c0, cw)...]
    gofs = np.zeros(NBR + 1, np.int64)      # base col per range
    c = 0
    for r in range(NBR):
        gofs[r] = c
        for w in range(NWIN):
            c0 = c
            for b in range(r * BRSZ, (r + 1) * BRSZ):
                for gi in range(g_wb[w, b]):
                    col_of[(w, b, gi)] = c
                    blockcols[b].append((c, w))
                    c += 1
            if c > c0:
                calls[r].append((w, c0, c - c0))
    gofs[NBR] = c
    GT = c                                   # total groups
    GMAX = int((gofs[1:] - gofs[:-1]).max())

    # per-core gather data
    idx16 = np.zeros((NCORES, 128, GT * 8), np.int16)
    dstloc = np.full((NCORES, 128, GT), 999.0, np.float16)
    normv = np.zeros((NCORES, 128, GT), np.float32)
    for k in range(NCORES):
        m = np.where(e_k == k)[0]
        order = m[np.lexsort((e_blk[m], e_w[m]))]
        ws = e_w[order]
        bs = e_blk[order]
        r16s = e_r16[order]
        dls = e_dloc[order]
        nms = e_norm[order]
        # within each (w,b) run, positions 0..cnt-1
        kw = ws * NBLK + bs
        chg = np.empty(kw.shape[0], np.bool_)
        chg[0] = True
        chg[1:] = kw[1:] != kw[:-1]
        starts = np.where(chg)[0]
        run_id = np.cumsum(chg) - 1
        pos = np.arange(kw.shape[0]) - starts[run_id]
        base_col = np.array([col_of[(w, b, 0)] for (w, b) in
                             zip(ws[starts], bs[starts])], np.int64)
        colv = base_col[run_id] + pos // 128
        j = pos % 128
        idx16[k][j % 16, colv * 8 + j // 16] = r16s
        dstloc[k][j, colv] = dls.astype(np.float16)
        normv[k][j, colv] = nms
        # pads keep idx 0 (valid row in window), dstloc 999 (no one-hot hit)
        idx16[k][16:, :] = np.tile(idx16[k][:16, :], (7, 1))

    # self-loop scale dinv^2, [128, NSLOT] natural order per core
    NSLOT = NSH // 128
    sn = (dinv * dinv).astype(np.float32)
    snorm = np.empty((NCORES, 128, NSLOT), np.float32)
    for k in range(NCORES):
        snorm[k] = sn[k * NSH:(k + 1) * NSH].reshape(NSLOT, 128).T

    # x transposed per core
    xT = np.empty((NCORES, 8, NSH), np.float32)
    for k in range(NCORES):
        xT[k] = x[k * NSH:(k + 1) * NSH].T

    # ---- final stage: identical structure on every core ----
    goff = (np.arange(GPC, dtype=np.int64) * NPG)[:, None]
    u = (goff + BRANCH_U[None, :]).reshape(-1)   # [NOUT] local rows
    v = (goff + BRANCH_V[None, :]).reshape(-1)
    cls = (u // FW0) * 2 + (v // FW0)
    order_f = np.argsort(cls, kind="stable")
    ccnt = np.bincount(cls, minlength=4)
    cpad = (np.ceil(ccnt / 128) * 128).astype(np.int64)
    NOUTP = int(cpad.sum())
    fu = np.zeros(NOUTP, np.int64)
    fv = np.zeros(NOUTP, np.int64)
    edge_ids = np.full(NOUTP, -1, np.int64)
    fcalls = []        # (uwin, vwin, group0, ngroups) per chunk call
    p0 = 0
    o0 = 0
    for cc in range(4):
        n = int(ccnt[cc])
        sel = order_f[o0:o0 + n]
        fu[p0:p0 + n] = u[sel]
        fv[p0:p0 + n] = v[sel]
        fu[p0 + n:p0 + int(cpad[cc])] = (cc // 2) * FW0
        fv[p0 + n:p0 + int(cpad[cc])] = (cc % 2) * FW0
        edge_ids[p0:p0 + n] = sel
        ngr = int(cpad[cc]) // 128
        g0 = p0 // 128
        for s in range(0, ngr, 16):
            fcalls.append((cc // 2, cc % 2, g0 + s, min(16, ngr - s)))
        p0 += int(cpad[cc])
        o0 += n
    fu16 = np.zeros((128, NOUTP // 16), np.int16)
    fv16 = np.zeros((128, NOUTP // 16), np.int16)
    j = np.arange(NOUTP)
    fu16[j % 16, j // 16] = (fu % FW0).astype(np.int16)
    fv16[j % 16, j // 16] = (fv % FW0).astype(np.int16)
    fu16[16:, :] = np.tile(fu16[:16, :], (7, 1))
    fv16[16:, :] = np.tile(fv16[:16, :], (7, 1))

    struct = dict(g_wb=g_wb, blockcols=blockcols, calls=calls, gofs=gofs,
                  GT=GT, GMAX=GMAX, fcalls=fcalls, NOUTP=NOUTP,
                  edge_ids=edge_ids)
    data = dict(idx16=idx16, dstloc=dstloc, normv=normv, snorm=snorm, xT=xT,
                fu16=fu16, fv16=fv16)
    return struct, data


def _consts(enc_W, enc_b, conv_W, conv_b, bn_gamma, bn_beta, bn_mean, bn_var,
            mlp_W1, mlp_b1, mlp_W2, mlp_b2):
    bnscale = (bn_gamma / np.sqrt(bn_var + BN_EPS)).astype(np.float32)
    bnshift = ((conv_b - bn_mean) * bnscale + bn_beta).astype(np.float32)
    consts = dict(
        encW=enc_W.astype(np.float32),                       # [8,128]
        encb=enc_b.reshape(128, 1).astype(np.float32),
        convW=np.concatenate([conv_W[i] for i in range(LAYERS)], axis=1
                             ).astype(np.float16),           # [128, 768]
        bnscale=bnscale.T.copy(),                            # [128, 6]
        bnshift=bnshift.T.copy(),
        w1u=mlp_W1[:128].astype(np.float16),
        w1v=mlp_W1[128:].astype(np.float16),
        w2=mlp_W2.astype(np.float16),                        # [128,1]
        b1=mlp_b1.reshape(128, 1).astype(np.float32),
    )
    b2 = float(np.asarray(mlp_b2).reshape(-1)[0])
    return consts, b2


def _build(struct, b2):
    GT = struct["GT"]
    GMAX = struct["GMAX"]
    gofs = struct["gofs"]
    calls = struct["calls"]
    blockcols = struct["blockcols"]
    fcalls = struct["fcalls"]
    NOUTP = struct["NOUTP"]
    NSLOT = NSH // 128
    NFCH = NOUTP // 2048 if NOUTP % 2048 == 0 else None

    nc = bacc.Bacc("TRN2", target_bir_lowering=False, debug=False,
                   num_devices=NCORES)

    xT_d = nc.dram_tensor("xT", [8, NSH], F32, kind="ExternalInput")
    idx16_d = nc.dram_tensor("idx16", [128, GT * 8], I16,
                             kind="ExternalInput")
    dstloc_d = nc.dram_tensor("dstloc", [128, GT], F16, kind="ExternalInput")
    norm_d = nc.dram_tensor("normv", [128, GT], F32, kind="ExternalInput")
    fu16_d = nc.dram_tensor("fu16", [128, NOUTP // 16], I16,
                            kind="ExternalInput")
    fv16_d = nc.dram_tensor("fv16", [128, NOUTP // 16], I16,
                            kind="ExternalInput")
    encW_d = nc.dram_tensor("encW", [8, 128], F32, kind="ExternalInput")
    encb_d = nc.dram_tensor("encb", [128, 1], F32, kind="ExternalInput")
    convW_d = nc.dram_tensor("convW", [128, LAYERS * 128], F16,
                             kind="ExternalInput")
    bnscale_d = nc.dram_tensor("bnscale", [128, LAYERS], F32,
                               kind="ExternalInput")
    bnshift_d = nc.dram_tensor("bnshift", [128, LAYERS], F32,
                               kind="ExternalInput")
    w1u_d = nc.dram_tensor("w1u", [128, 128], F16, kind="ExternalInput")
    w1v_d = nc.dram_tensor("w1v", [128, 128], F16, kind="ExternalInput")
    w2_d = nc.dram_tensor("w2", [128, 1], F16, kind="ExternalInput")
    b1_d = nc.dram_tensor("b1", [128, 1], F32, kind="ExternalInput")
    snorm_d = nc.dram_tensor("snorm", [128, NSLOT], F32, kind="ExternalInput")

    out_d = nc.dram_tensor("outd", [1, NOUTP], F32, kind="ExternalOutput")

    hloc = [[nc.dram_tensor(f"hloc{j}_{c}", [CHSH, 128], F16,
                            kind="Internal") for c in range(SPLIT)]
            for j in range(2)]
    hloc6 = nc.dram_tensor("hloc6", [NSH, 128], F16, kind="Internal")
    tab = [nc.dram_tensor(f"tab{j}", [N, 128], F16, kind="Internal",
                          addr_space="Shared") for j in range(2)]
    RG = [list(range(NCORES))]

    def rows3(t, b):
        return t[b * 512:(b + 1) * 512, :].rearrange("(j p) f -> p j f", p=128)

    def rows3c(hl, b):
        return rows3(hl[b // BPC], b % BPC)

    with tile.TileContext(nc) as tc:
        with (
            tc.tile_pool(name="const", bufs=1) as cpool,
            tc.tile_pool(name="gat", bufs=2) as gp,
            tc.tile_pool(name="fgat", bufs=2) as fgp,
            tc.tile_pool(name="sone", bufs=4) as sp,
            tc.tile_pool(name="mid", bufs=2) as mp,
            tc.tile_pool(name="pps", bufs=2, space="PSUM") as pps,
            tc.tile_pool(name="ppy", bufs=2, space="PSUM") as ppy,
            tc.tile_pool(name="ppt", bufs=2, space="PSUM") as ppt,
        ):
            # ---- constants ----
            iota_i = cpool.tile([128, 128], I16, tag="iotai")
            nc.gpsimd.iota(iota_i[:], pattern=[[1, 128]], base=0,
                           channel_multiplier=0)
            prow_i = cpool.tile([128, 128], I16, tag="prowi")
            nc.gpsimd.iota(prow_i[:], pattern=[[0, 128]], base=0,
                           channel_multiplier=1)
            ident32 = cpool.tile([128, 128], F32, tag="id32")
            nc.vector.tensor_tensor(out=ident32[:], in0=prow_i[:],
                                    in1=iota_i[:], op=mybir.AluOpType.is_equal)
            ident16 = cpool.tile([128, 128], F16, tag="id16")
            nc.vector.tensor_copy(out=ident16[:], in_=ident32[:])
            iota5_i = cpool.tile([128, 512], I16, tag="iota5i")
            nc.gpsimd.iota(iota5_i[:], pattern=[[1, 512]], base=0,
                           channel_multiplier=0)
            iota512 = cpool.tile([128, 512], F16, tag="iota512")
            nc.vector.tensor_copy(out=iota512[:], in_=iota5_i[:])

            idx16_sb = cpool.tile([128, GT * 8], I16, tag="idx16")
            nc.sync.dma_start(out=idx16_sb[:], in_=idx16_d[:, :])
            dstloc_sb = cpool.tile([128, GT], F16, tag="dstloc")
            nc.sync.dma_start(out=dstloc_sb[:], in_=dstloc_d[:, :])
            norm_sb = cpool.tile([128, GT], F32, tag="norm")
            nc.sync.dma_start(out=norm_sb[:], in_=norm_d[:, :])
            fu16_sb = cpool.tile([128, NOUTP // 16], I16, tag="fu16")
            nc.sync.dma_start(out=fu16_sb[:], in_=fu16_d[:, :])
            fv16_sb = cpool.tile([128, NOUTP // 16], I16, tag="fv16")
            nc.sync.dma_start(out=fv16_sb[:], in_=fv16_d[:, :])
            encW_sb = cpool.tile([8, 128], F32, tag="encW")
            nc.sync.dma_start(out=encW_sb[:], in_=encW_d[:, :])
            encb_sb = cpool.tile([128, 1], F32, tag="encb")
            nc.sync.dma_start(out=encb_sb[:], in_=encb_d[:, :])
            convW_sb = cpool.tile([128, LAYERS * 128], F16, tag="convW")
            nc.sync.dma_start(out=convW_sb[:], in_=convW_d[:, :])
            bnscale_sb = cpool.tile([128, LAYERS], F32, tag="bns")
            nc.sync.dma_start(out=bnscale_sb[:], in_=bnscale_d[:, :])
            bnshift_sb = cpool.tile([128, LAYERS], F32, tag="bnsh")
            nc.sync.dma_start(out=bnshift_sb[:], in_=bnshift_d[:, :])
            w1u_sb = cpool.tile([128, 128], F16, tag="w1u")
            nc.sync.dma_start(out=w1u_sb[:], in_=w1u_d[:, :])
            w1v_sb = cpool.tile([128, 128], F16, tag="w1v")
            nc.sync.dma_start(out=w1v_sb[:], in_=w1v_d[:, :])
            w2_sb = cpool.tile([128, 1], F16, tag="w2")
            nc.sync.dma_start(out=w2_sb[:], in_=w2_d[:, :])
            b1_sb = cpool.tile([128, 1], F32, tag="b1")
            nc.sync.dma_start(out=b1_sb[:], in_=b1_d[:, :])
            snorm_sb = cpool.tile([128, NSLOT], F32, tag="snorm")
            nc.sync.dma_start(out=snorm_sb[:], in_=snorm_d[:, :])

            def transpose_store(y_sb, rsd, dst_rows3, with_res):
                psum_t = ppt.tile([128, 4, 128], F32, tag="pt")
                for j in range(4):
                    nc.tensor.matmul(
                        out=psum_t[:, j, :],
                        lhsT=y_sb[:, j * 128:(j + 1) * 128],
                        rhs=ident32[:], is_transpose=True,
                        start=True, stop=(not with_res),
                        skip_group_check=True)
                    if with_res:
                        nc.tensor.matmul(
                            out=psum_t[:, j, :], lhsT=ident16[:],
                            rhs=rsd[:, j, :], start=False, stop=True,
                            skip_group_check=True)
                t16 = mp.tile([128, 4, 128], F16, tag="t16")
                nc.vector.tensor_copy(out=t16[:], in_=psum_t[:])
                nc.sync.dma_start(out=dst_rows3, in_=t16[:])

            def ag_chunk(li, c):
                src = hloc[li % 2][c]
                dstt = tab[(li + 1) % 2]
                nc.gpsimd.collective_compute(
                    "AllGather", mybir.AluOpType.bypass,
                    replica_groups=RG,
                    ins=[src[:, :]],
                    outs=[dstt[c * CHSH * NCORES:(c + 1) * CHSH * NCORES, :]],
                )

            # ---- encoder: h0 = x @ encW + encb -> hloc[1] ----
            with nc.named_scope("encoder"):
                for b in range(NBLK):
                    xt = mp.tile([8, 512], F32, tag="xt")
                    nc.sync.dma_start(out=xt[:],
                                      in_=xT_d[:, b * 512:(b + 1) * 512])
                    psum_y = ppy.tile([128, 512], F32, tag="py")
                    nc.tensor.matmul(out=psum_y[:], lhsT=encW_sb[:],
                                     rhs=xt[:], start=True, stop=True)
                    y_sb = mp.tile([128, 512], F32, tag="y_sb")
                    nc.vector.tensor_scalar_add(out=y_sb[:], in0=psum_y[:],
                                                scalar1=encb_sb[:, :])
                    transpose_store(y_sb, None, rows3c(hloc[1], b), False)
                    if (b + 1) % BPC == 0:
                        ag_chunk(-1, b // BPC)

            # ---- 6 GCN layers ----
            import os as _os2
            _nlayers = int(_os2.environ.get("KERNEL_NLAYERS", str(LAYERS)))
            _nbrlim = int(_os2.environ.get("KERNEL_NBR_LIM", str(NBR)))
            for li in range(_nlayers):
                t_cur = tab[li % 2]
                h_prev = hloc[(li + 1) % 2]
                h_next = hloc[li % 2]
                last = (li == LAYERS - 1)
                with nc.named_scope(f"layer{li}"):
                    for r in range(_nbrlim):
                        g0 = int(gofs[r])
                        G_r = int(gofs[r + 1]) - g0
                        Msup = gp.tile([128, GMAX, 128], F16, tag="Msup")
                        if not _os2.environ.get("KERNEL_NO_GATHER"):
                            for (w, c0, cw) in calls[r]:
                                nc.gpsimd.dma_gather(
                                    out_ap=Msup[:, c0 - g0:c0 - g0 + cw, :],
                                    in_ap=t_cur[w * W0:(w + 1) * W0, :],
                                    idxs_ap=idx16_sb[:, c0 * 8:(c0 + cw) * 8],
                                    num_idxs=cw * 128, num_idxs_reg=cw * 128,
                                    elem_size=128,
                                    single_packet=not _os2.environ.get(
                                        "KERNEL_MULTI_PACKET"))
                        else:
                            nc.vector.memset(Msup[:, :, :], 0.25)
                        nb = norm_sb[:, g0:g0 + G_r][:, :, None] \
                            .to_broadcast([128, G_r, 128])
                        nc.vector.tensor_tensor(
                            out=Msup[:, :G_r, :], in0=Msup[:, :G_r, :],
                            in1=nb, op=mybir.AluOpType.mult)
                        if _os2.environ.get("KERNEL_GATHER_ONLY"):
                            continue
                        for b in range(r * BRSZ, (r + 1) * BRSZ):
                            psum_s = pps.tile([128, 512], F32, tag="ps")
                            rsd = mp.tile([128, 4, 128], F16, tag="rsd")
                            nc.sync.dma_start(out=rsd[:],
                                              in_=rows3c(h_prev, b))
                            first = True
                            for (cc, w) in blockcols[b]:
                                S = sp.tile([128, 512], F16, tag="S")
                                nc.vector.tensor_tensor(
                                    out=S[:],
                                    in0=dstloc_sb[:, cc:cc + 1]
                                    .to_broadcast([128, 512]),
                                    in1=iota512[:],
                                    op=mybir.AluOpType.is_equal)
                                nc.tensor.matmul(
                                    out=psum_s[:], lhsT=Msup[:, cc - g0, :],
                                    rhs=S[:], start=first, stop=False,
                                    skip_group_check=True)
                                first = False
                            for q in range(4):
                                Ssl = sp.tile([128, 128], F16, tag="Ssl")
                                nc.vector.tensor_scalar_mul(
                                    out=Ssl[:], in0=ident16[:],
                                    scalar1=snorm_sb[:,
                                                     b * 4 + q:b * 4 + q + 1])
                                nc.tensor.matmul(
                                    out=psum_s[:, q * 128:(q + 1) * 128],
                                    lhsT=rsd[:, q, :], rhs=Ssl[:],
                                    start=first, stop=True,
                                    skip_group_check=True)
                            first = False
                            s_sb = mp.tile([128, 512], F16, tag="s_sb")
                            nc.vector.tensor_copy(out=s_sb[:], in_=psum_s[:])
                            psum_y = ppy.tile([128, 512], F32, tag="py")
                            nc.tensor.matmul(
                                out=psum_y[:],
                                lhsT=convW_sb[:, li * 128:(li + 1) * 128],
                                rhs=s_sb[:], start=True, stop=True)
                            y_sb = mp.tile([128, 512], F32, tag="y_sb")
                            nc.scalar.activation(
                                out=y_sb[:], in_=psum_y[:],
                                func=mybir.ActivationFunctionType.Relu,
                                bias=bnshift_sb[:, li:li + 1],
                                scale=bnscale_sb[:, li:li + 1])
                            transpose_store(
                                y_sb, rsd,
                                rows3(hloc6, b) if last else rows3c(h_next, b),
                                True)
                            if (not last and (b + 1) % BPC == 0
                                    and not _os2.environ.get("KERNEL_NO_AG")):
                                ag_chunk(li, b // BPC)

            # ---- final edge MLP ----
            import os as _os
            _skip_final = bool(int(_os.environ.get("SKIP_FINAL", "0")))
            h6 = hloc6
            fwin = [h6[0:FW0, :], h6[FW0:NSH, :]]
            with nc.named_scope("final"):
                for (uw, vw, gr0, ngr) in ([] if _skip_final else fcalls):
                    ehu = fgp.tile([128, 16, 128], F16, tag="ehu")
                    nc.gpsimd.dma_gather(
                        out_ap=ehu[:, 0:ngr, :], in_ap=fwin[uw],
                        idxs_ap=fu16_sb[:, gr0 * 8:(gr0 + ngr) * 8],
                        num_idxs=ngr * 128, num_idxs_reg=ngr * 128,
                        elem_size=128, single_packet=False)
                    ehv = fgp.tile([128, 16, 128], F16, tag="ehv")
                    nc.gpsimd.dma_gather(
                        out_ap=ehv[:, 0:ngr, :], in_ap=fwin[vw],
                        idxs_ap=fv16_sb[:, gr0 * 8:(gr0 + ngr) * 8],
                        num_idxs=ngr * 128, num_idxs_reg=ngr * 128,
                        elem_size=128, single_packet=False)
                    hid16 = fgp.tile([128, 16, 128], F16, tag="hid16")
                    for tt in range(ngr):
                        psum_e = ppt.tile([128, 2, 128], F16, tag="pt16")
                        nc.tensor.matmul(out=psum_e[:, 0, :],
                                         lhsT=ehu[:, tt, :],
                                         rhs=ident16[:], is_transpose=True,
                                         start=True, stop=True,
                                         skip_group_check=True)
                        nc.tensor.matmul(out=psum_e[:, 1, :],
                                         lhsT=ehv[:, tt, :],
                                         rhs=ident16[:], is_transpose=True,
                                         start=True, stop=True,
                                         skip_group_check=True)
                        ehT = mp.tile([128, 2, 128], F16, tag="ehT")
                        nc.vector.tensor_copy(out=ehT[:], in_=psum_e[:])
                        psum_h = pps.tile([128, 128], F32, tag="ps")
                        nc.tensor.matmul(out=psum_h[:], lhsT=w1u_sb[:],
                                         rhs=ehT[:, 0, :], start=True,
                                         stop=False)
                        nc.tensor.matmul(out=psum_h[:], lhsT=w1v_sb[:],
                                         rhs=ehT[:, 1, :], start=False,
                                         stop=True)
                        nc.scalar.activation(
                            out=hid16[:, tt, :], in_=psum_h[:],
                            func=mybir.ActivationFunctionType.Relu,
                            bias=b1_sb[:, :], scale=1.0)
                    o_sb = fgp.tile([1, 2048], F32, tag="o_sb")
                    for half in range(4):
                        if half * 4 >= ngr:
                            break
                        nh = min(4, ngr - half * 4)
                        psum_o = ppy.tile([1, 512], F32, tag="py")
                        nc.tensor.matmul(
                            out=psum_o[:, :nh * 128], lhsT=w2_sb[:],
                            rhs=hid16[:, 4 * half:4 * half + nh, :].rearrange(
                                "p t f -> p (t f)"),
                            start=True, stop=True)
                        nc.vector.tensor_scalar_add(
                            out=o_sb[:, 512 * half:512 * half + nh * 128],
                            in0=psum_o[:, :nh * 128], scalar1=b2)
                    nc.sync.dma_start(
                        out=out_d[0:1, gr0 * 128:(gr0 + ngr) * 128],
                        in_=o_sb[:, :ngr * 128])

    nc.finalize()
    return nc


def kernel(**inputs):
    x = np.asarray(inputs["x"], np.float32)
    edge_index = np.asarray(inputs["edge_index"])
    assert x.shape == (N, 8)
    struct, data = _prep(x, edge_index)
    consts, b2 = _consts(
        np.asarray(inputs["enc_W"], np.float32),
        np.asarray(inputs["enc_b"], np.float32),
        np.asarray(inputs["conv_W"], np.float32),
        np.asarray(inputs["conv_b"], np.float32),
        np.asarray(inputs["bn_gamma"], np.float32),
        np.asarray(inputs["bn_beta"], np.float32),
        np.asarray(inputs["bn_mean"], np.float32),
        np.asarray(inputs["bn_var"], np.float32),
        np.asarray(inputs["mlp_W1"], np.float32),
        np.asarray(inputs["mlp_b1"], np.float32),
        np.asarray(inputs["mlp_W2"], np.float32),
        np.asarray(inputs["mlp_b2"], np.float32))

    key = "nc"
    if key not in _CACHE:
        _CACHE[key] = _build(struct, b2)
    nc = _CACHE[key]

    in_maps = []
    for k in range(NCORES):
        in_maps.append({
            "xT": data["xT"][k], "idx16": data["idx16"][k],
            "dstloc": data["dstloc"][k], "normv": data["normv"][k],
            "snorm": data["snorm"][k],
            "fu16": data["fu16"], "fv16": data["fv16"],
            "encW": consts["encW"], "encb": consts["encb"],
            "convW": consts["convW"], "bnscale": consts["bnscale"],
            "bnshift": consts["bnshift"], "w1u": consts["w1u"],
            "w1v": consts["w1v"], "w2": consts["w2"], "b1": consts["b1"],
        })

    trace = bool(int(__import__("os").environ.get("KERNEL_TRACE", "0")))
    res = run_bass_kernel_spmd(nc, in_maps, core_ids=list(range(NCORES)),
                               trace=trace)
    kernel.last_result = res
    edge_ids = struct["edge_ids"]
    valid = edge_ids >= 0
    out = np.empty((NCORES, NOUT), np.float32)
    for k in range(NCORES):
        r = np.asarray(res.results[k]["outd"], np.float32).reshape(-1)
        out[k][edge_ids[valid]] = r[valid]
    return out.reshape(NCORES * NOUT, 1)


# revision 33
# speedup vs baseline: 1.0228x; 1.0228x over previous
"""Trainium2 Bass kernel for nn_DeepEdgeCongestionGNN (6-layer GCN + edge MLP).

Strategy (8 NeuronCores, SPMD):
  - Nodes sharded by graph (2048 graphs = 61440 nodes per core), natural
    order. Per layer the full fp16 node table (chunk-interleaved rows) is
    AllGather-replicated; each core gathers the rows its edges need with
    dma_gather (int16 in-window indices; the 491520-row table splits into
    15 windows of 32768 rows).
  - Entries are sorted by (window, dst 512-node block) and padded to
    128-entry groups that are window-pure and block-pure. One dma_gather
    per (window, 8-block range) fetches ~1024 rows; a 512-wide one-hot
    matmul scatters each group into the block's PSUM bank (feat x node),
    self-loops enter via identity matmuls on the residual tile.
  - y^T = W^T s^T, fused BN+ReLU on ACT, transpose back node-major with
    the residual added on the PE, store shard, AllGather chunks.
  - Final edge MLP gathers h6[u],h6[v] via dma_gather (edges sorted by
    (u-window, v-window) class); host un-permutes the padded output.
  - The bass program is built per input (structure depends on edge counts)
    but is identical on all cores: group counts take the max over cores.
"""
import sys
import types

import numpy as np

sys.path.insert(0, "/opt/trn_rl_repo")

# --- shim antenv.axon_hooks (absent in this image) so trace=True works ---
import antenv
if "antenv.axon_hooks" not in sys.modules:
    _hookmod = types.ModuleType("antenv.axon_hooks")
    _hookmod._hook = None
    def _set(h): _hookmod._hook = h
    def _get(): return _hookmod._hook
    _hookmod.set_axon_ntff_profile_hook = _set
    _hookmod.get_axon_ntff_profile_hook = _get
    sys.modules["antenv.axon_hooks"] = _hookmod
    antenv.axon_hooks = _hookmod
    try:
        from trn_agent_boot.trn_boot import _ntff_profile_via_ctypes
        _hookmod._hook = _ntff_profile_via_ctypes("/opt/axon/libaxon_pjrt.so")
    except Exception:
        pass

import concourse.bass as bass
import concourse.bacc as bacc
import concourse.mybir as mybir
import concourse.tile as tile
from concourse.bass_utils import run_bass_kernel_spmd

F16 = mybir.dt.float16
F32 = mybir.dt.float32
I32 = mybir.dt.int32
I16 = mybir.dt.int16

NCORES = 8
NPG = 30                    # nodes per graph
G = 16384                   # graphs
N = G * NPG                 # 491520 nodes
GPC = G // NCORES           # graphs per core
NSH = GPC * NPG             # 61440 nodes per core
NBLK = NSH // 512           # 120 psum blocks of 512 nodes
LAYERS = 6
HID = 128
SPLIT = 4                   # AllGather chunks per layer
CHSH = NSH // SPLIT         # shard rows per AG chunk
BPC = NBLK // SPLIT         # blocks per AG chunk
W0 = 32768                  # dma_gather window rows (int16 idx range)
NWIN = N // W0              # 15 table windows
BRSZ = 8                    # blocks per gather range
NBR = NBLK // BRSZ          # 15 ranges
FW0 = 32768                 # final-stage window over NSH rows
NFW = 2                     # final windows (61440 = 32768 + 28672)
NOUT = GPC * 41             # 83968 output rows per core
BN_EPS = 1e-5

BRANCH_U = np.array([0,0,1,2,1,1,3,5,5,6,6,6,6,8,8,9,11,11,11,11,13,15,14,17,
                     18,9,9,21,14,21,22,23,24,24,27,26,26,28,26,7,5],
                    dtype=np.int64)
BRANCH_V = np.array([1,2,3,3,4,5,5,6,7,7,8,9,27,9,10,10,12,13,15,16,14,16,17,
                     18,19,19,20,20,22,21,23,23,24,26,26,29,28,29,27,27,8],
                    dtype=np.int64)

_CACHE = {}


def _table_row(g):
    """Global node id -> row in the chunk-interleaved AllGather table."""
    k = g // NSH
    l = g % NSH
    return (l // CHSH) * (CHSH * NCORES) + k * CHSH + (l % CHSH)


def _prep(x, edge_index):
    src = np.ascontiguousarray(edge_index[0]).astype(np.int64)
    dst = np.ascontiguousarray(edge_index[1]).astype(np.int64)

    indeg = np.bincount(dst, minlength=N).astype(np.int64)
    deg = (indeg + 1).astype(np.float64)
    dinv = (1.0 / np.sqrt(deg)).astype(np.float32)

    e_k = dst // NSH                       # consumer core
    e_l = dst % NSH
    e_blk = (e_l // 512).astype(np.int64)
    e_dloc = (e_l % 512).astype(np.int64)
    srow = _table_row(src)
    e_w = srow // W0
    e_r16 = (srow % W0).astype(np.int64)
    e_norm = (dinv[src] * dinv[dst]).astype(np.float32)

    # counts per (core, window, block)
    key_wb = e_w * NBLK + e_blk
    cnt = np.zeros((NCORES, NWIN * NBLK), np.int64)
    for k in range(NCORES):
        m = e_k == k
        cnt[k] = np.bincount(key_wb[m], minlength=NWIN * NBLK)
    g_wb = np.ceil(cnt.max(axis=0) / 128).astype(np.int64)  # shared structure
    g_wb = g_wb.reshape(NWIN, NBLK)

    # column enumeration: range-major, then window, then block, then group
    col_of = {}
    blockcols = [[] for _ in range(NBLK)]   # per block: [(col, w)...]
    calls = [[] for _ in range(NBR)]        # per range: [(w, c0, cw)...]
    gofs = np.zeros(NBR + 1, np.int64)      # base col per range
    c = 0
    for r in range(NBR):
        gofs[r] = c
        for w in range(NWIN):
            c0 = c
            for b in range(r * BRSZ, (r + 1) * BRSZ):
                for gi in range(g_wb[w, b]):
                    col_of[(w, b, gi)] = c
                    blockcols[b].append((c, w))
                    c += 1
            if c > c0:
                calls[r].append((w, c0, c - c0))
    gofs[NBR] = c
    GT = c                                   # total groups
    GMAX = int((gofs[1:] - gofs[:-1]).max())

    # per-core gather data
    idx16 = np.zeros((NCORES, 128, GT * 8), np.int16)
    dstloc = np.full((NCORES, 128, GT), 999.0, np.float32)
    normv = np.zeros((NCORES, 128, GT), np.float32)
    for k in range(NCORES):
        m = np.where(e_k == k)[0]
        order = m[np.lexsort((e_blk[m], e_w[m]))]
        ws = e_w[order]
        bs = e_blk[order]
        r16s = e_r16[order]
        dls = e_dloc[order]
        nms = e_norm[order]
        # within each (w,b) run, positions 0..cnt-1
        kw = ws * NBLK + bs
        chg = np.empty(kw.shape[0], np.bool_)
        chg[0] = True
        chg[1:] = kw[1:] != kw[:-1]
        starts = np.where(chg)[0]
        run_id = np.cumsum(chg) - 1
        pos = np.arange(kw.shape[0]) - starts[run_id]
        base_col = np.array([col_of[(w, b, 0)] for (w, b) in
                             zip(ws[starts], bs[starts])], np.int64)
        colv = base_col[run_id] + pos // 128
        j = pos % 128
        idx16[k][j % 16, colv * 8 + j // 16] = r16s
        dstloc[k][j, colv] = dls.astype(np.float32)
        normv[k][j, colv] = nms
        # pads keep idx 0 (valid row in window), dstloc 999 (no one-hot hit)
        idx16[k][16:, :] = np.tile(idx16[k][:16, :], (7, 1))

    # self-loop scale dinv^2, [128, NSLOT] natural order per core
    NSLOT = NSH // 128
    sn = (dinv * dinv).astype(np.float32)
    snorm = np.empty((NCORES, 128, NSLOT), np.float32)
    for k in range(NCORES):
        snorm[k] = sn[k * NSH:(k + 1) * NSH].reshape(NSLOT, 128).T

    # x transposed per core
    xT = np.empty((NCORES, 8, NSH), np.float32)
    for k in range(NCORES):
        xT[k] = x[k * NSH:(k + 1) * NSH].T

    # ---- final stage: identical structure on every core ----
    goff = (np.arange(GPC, dtype=np.int64) * NPG)[:, None]
    u = (goff + BRANCH_U[None, :]).reshape(-1)   # [NOUT] local rows
    v = (goff + BRANCH_V[None, :]).reshape(-1)
    cls = (u // FW0) * 2 + (v // FW0)
    order_f = np.argsort(cls, kind="stable")
    ccnt = np.bincount(cls, minlength=4)
    cpad = (np.ceil(ccnt / 128) * 128).astype(np.int64)
    NOUTP = int(cpad.sum())
    fu = np.zeros(NOUTP, np.int64)
    fv = np.zeros(NOUTP, np.int64)
    edge_ids = np.full(NOUTP, -1, np.int64)
    fcalls = []        # (uwin, vwin, group0, ngroups) per chunk call
    p0 = 0
    o0 = 0
    for cc in range(4):
        n = int(ccnt[cc])
        sel = order_f[o0:o0 + n]
        fu[p0:p0 + n] = u[sel]
        fv[p0:p0 + n] = v[sel]
        fu[p0 + n:p0 + int(cpad[cc])] = (cc // 2) * FW0
        fv[p0 + n:p0 + int(cpad[cc])] = (cc % 2) * FW0
        edge_ids[p0:p0 + n] = sel
        ngr = int(cpad[cc]) // 128
        g0 = p0 // 128
        for s in range(0, ngr, 16):
            fcalls.append((cc // 2, cc % 2, g0 + s, min(16, ngr - s)))
        p0 += int(cpad[cc])
        o0 += n
    fu16 = np.zeros((128, NOUTP // 16), np.int16)
    fv16 = np.zeros((128, NOUTP // 16), np.int16)
    j = np.arange(NOUTP)
    fu16[j % 16, j // 16] = (fu % FW0).astype(np.int16)
    fv16[j % 16, j // 16] = (fv % FW0).astype(np.int16)
    fu16[16:, :] = np.tile(fu16[:16, :], (7, 1))
    fv16[16:, :] = np.tile(fv16[:16, :], (7, 1))

    struct = dict(g_wb=g_wb, blockcols=blockcols, calls=calls, gofs=gofs,
                  GT=GT, GMAX=GMAX, fcalls=fcalls, NOUTP=NOUTP,
                  edge_ids=edge_ids)
    data = dict(idx16=idx16, dstloc=dstloc, normv=normv, snorm=snorm, xT=xT,
                fu16=fu16, fv16=fv16)
    return struct, data


def _consts(enc_W, enc_b, conv_W, conv_b, bn_gamma, bn_beta, bn_mean, bn_var,
            mlp_W1, mlp_b1, mlp_W2, mlp_b2):
    bnscale = (bn_gamma / np.sqrt(bn_var + BN_EPS)).astype(np.float32)
    bnshift = ((conv_b - bn_mean) * bnscale + bn_beta).astype(np.float32)
    consts = dict(
        encW=enc_W.astype(np.float32),                       # [8,128]
        encb=enc_b.reshape(128, 1).astype(np.float32),
        convW=np.concatenate([conv_W[i] for i in range(LAYERS)], axis=1
                             ).astype(np.float16),           # [128, 768]
        bnscale=bnscale.T.copy(),                            # [128, 6]
        bnshift=bnshift.T.copy(),
        w1u=mlp_W1[:128].astype(np.float16),
        w1v=mlp_W1[128:].astype(np.float16),
        w2=mlp_W2.astype(np.float16),                        # [128,1]
        b1=mlp_b1.reshape(128, 1).astype(np.float32),
    )
    b2 = float(np.asarray(mlp_b2).reshape(-1)[0])
    return consts, b2


def _build(struct, b2):
    GT = struct["GT"]
    GMAX = struct["GMAX"]
    gofs = struct["gofs"]
    calls = struct["calls"]
    blockcols = struct["blockcols"]
    fcalls = struct["fcalls"]
    NOUTP = struct["NOUTP"]
    NSLOT = NSH // 128
    NFCH = NOUTP // 2048 if NOUTP % 2048 == 0 else None

    nc = bacc.Bacc("TRN2", target_bir_lowering=False, debug=False,
                   num_devices=NCORES)

    xT_d = nc.dram_tensor("xT", [8, NSH], F32, kind="ExternalInput")
    idx16_d = nc.dram_tensor("idx16", [128, GT * 8], I16,
                             kind="ExternalInput")
    dstloc_d = nc.dram_tensor("dstloc", [128, GT], F32, kind="ExternalInput")
    norm_d = nc.dram_tensor("normv", [128, GT], F32, kind="ExternalInput")
    fu16_d = nc.dram_tensor("fu16", [128, NOUTP // 16], I16,
                            kind="ExternalInput")
    fv16_d = nc.dram_tensor("fv16", [128, NOUTP // 16], I16,
                            kind="ExternalInput")
    encW_d = nc.dram_tensor("encW", [8, 128], F32, kind="ExternalInput")
    encb_d = nc.dram_tensor("encb", [128, 1], F32, kind="ExternalInput")
    convW_d = nc.dram_tensor("convW", [128, LAYERS * 128], F16,
                             kind="ExternalInput")
    bnscale_d = nc.dram_tensor("bnscale", [128, LAYERS], F32,
                               kind="ExternalInput")
    bnshift_d = nc.dram_tensor("bnshift", [128, LAYERS], F32,
                               kind="ExternalInput")
    w1u_d = nc.dram_tensor("w1u", [128, 128], F16, kind="ExternalInput")
    w1v_d = nc.dram_tensor("w1v", [128, 128], F16, kind="ExternalInput")
    w2_d = nc.dram_tensor("w2", [128, 1], F16, kind="ExternalInput")
    b1_d = nc.dram_tensor("b1", [128, 1], F32, kind="ExternalInput")
    snorm_d = nc.dram_tensor("snorm", [128, NSLOT], F32, kind="ExternalInput")

    out_d = nc.dram_tensor("outd", [1, NOUTP], F32, kind="ExternalOutput")

    hloc = [[nc.dram_tensor(f"hloc{j}_{c}", [CHSH, 128], F16,
                            kind="Internal") for c in range(SPLIT)]
            for j in range(2)]
    hloc6 = nc.dram_tensor("hloc6", [NSH, 128], F16, kind="Internal")
    tab = [nc.dram_tensor(f"tab{j}", [N, 128], F16, kind="Internal",
                          addr_space="Shared") for j in range(2)]
    RG = [list(range(NCORES))]

    def rows3(t, b):
        return t[b * 512:(b + 1) * 512, :].rearrange("(j p) f -> p j f", p=128)

    def rows3c(hl, b):
        return rows3(hl[b // BPC], b % BPC)

    with tile.TileContext(nc) as tc:
        with (
            tc.tile_pool(name="const", bufs=1) as cpool,
            tc.tile_pool(name="gat",
                         bufs=int(__import__("os").environ.get(
                             "KERNEL_GP_BUFS", "2"))) as gp,
            tc.tile_pool(name="fgat", bufs=2) as fgp,
            tc.tile_pool(name="sone", bufs=4) as sp,
            tc.tile_pool(name="mid", bufs=2) as mp,
            tc.tile_pool(name="pps", bufs=2, space="PSUM") as pps,
            tc.tile_pool(name="ppy", bufs=2, space="PSUM") as ppy,
            tc.tile_pool(name="ppt", bufs=2, space="PSUM") as ppt,
        ):
            # ---- constants ----
            iota_i = cpool.tile([128, 128], I16, tag="iotai")
            nc.gpsimd.iota(iota_i[:], pattern=[[1, 128]], base=0,
                           channel_multiplier=0)
            prow_i = cpool.tile([128, 128], I16, tag="prowi")
            nc.gpsimd.iota(prow_i[:], pattern=[[0, 128]], base=0,
                           channel_multiplier=1)
            ident32 = cpool.tile([128, 128], F32, tag="id32")
            nc.vector.tensor_tensor(out=ident32[:], in0=prow_i[:],
                                    in1=iota_i[:], op=mybir.AluOpType.is_equal)
            ident16 = cpool.tile([128, 128], F16, tag="id16")
            nc.vector.tensor_copy(out=ident16[:], in_=ident32[:])
            iota5_i = cpool.tile([128, 512], I16, tag="iota5i")
            nc.gpsimd.iota(iota5_i[:], pattern=[[1, 512]], base=0,
                           channel_multiplier=0)
            iota512 = cpool.tile([128, 512], F16, tag="iota512")
            nc.vector.tensor_copy(out=iota512[:], in_=iota5_i[:])

            idx16_sb = cpool.tile([128, GT * 8], I16, tag="idx16")
            nc.sync.dma_start(out=idx16_sb[:], in_=idx16_d[:, :])
            dstloc_sb = cpool.tile([128, GT], F32, tag="dstloc")
            nc.sync.dma_start(out=dstloc_sb[:], in_=dstloc_d[:, :])
            norm_sb = cpool.tile([128, GT], F32, tag="norm")
            nc.sync.dma_start(out=norm_sb[:], in_=norm_d[:, :])
            fu16_sb = cpool.tile([128, NOUTP // 16], I16, tag="fu16")
            nc.sync.dma_start(out=fu16_sb[:], in_=fu16_d[:, :])
            fv16_sb = cpool.tile([128, NOUTP // 16], I16, tag="fv16")
            nc.sync.dma_start(out=fv16_sb[:], in_=fv16_d[:, :])
            encW_sb = cpool.tile([8, 128], F32, tag="encW")
            nc.sync.dma_start(out=encW_sb[:], in_=encW_d[:, :])
            encb_sb = cpool.tile([128, 1], F32, tag="encb")
            nc.sync.dma_start(out=encb_sb[:], in_=encb_d[:, :])
            convW_sb = cpool.tile([128, LAYERS * 128], F16, tag="convW")
            nc.sync.dma_start(out=convW_sb[:], in_=convW_d[:, :])
            bnscale_sb = cpool.tile([128, LAYERS], F32, tag="bns")
            nc.sync.dma_start(out=bnscale_sb[:], in_=bnscale_d[:, :])
            bnshift_sb = cpool.tile([128, LAYERS], F32, tag="bnsh")
            nc.sync.dma_start(out=bnshift_sb[:], in_=bnshift_d[:, :])
            w1u_sb = cpool.tile([128, 128], F16, tag="w1u")
            nc.sync.dma_start(out=w1u_sb[:], in_=w1u_d[:, :])
            w1v_sb = cpool.tile([128, 128], F16, tag="w1v")
            nc.sync.dma_start(out=w1v_sb[:], in_=w1v_d[:, :])
            w2_sb = cpool.tile([128, 1], F16, tag="w2")
            nc.sync.dma_start(out=w2_sb[:], in_=w2_d[:, :])
            b1_sb = cpool.tile([128, 1], F32, tag="b1")
            nc.sync.dma_start(out=b1_sb[:], in_=b1_d[:, :])
            snorm_sb = cpool.tile([128, NSLOT], F32, tag="snorm")
            nc.sync.dma_start(out=snorm_sb[:], in_=snorm_d[:, :])

            def transpose_store(y_sb, rsd, dst_rows3, with_res):
                psum_t = ppt.tile([128, 4, 128], F32, tag="pt")
                for j in range(4):
                    nc.tensor.matmul(
                        out=psum_t[:, j, :],
                        lhsT=y_sb[:, j * 128:(j + 1) * 128],
                        rhs=ident32[:], is_transpose=True,
                        start=True, stop=(not with_res),
                        skip_group_check=True)
                    if with_res:
                        nc.tensor.matmul(
                            out=psum_t[:, j, :], lhsT=ident16[:],
                            rhs=rsd[:, j, :], start=False, stop=True,
                            skip_group_check=True)
                t16 = mp.tile([128, 4, 128], F16, tag="t16")
                nc.scalar.activation(out=t16[:], in_=psum_t[:],
                                     func=mybir.ActivationFunctionType.Copy)
                nc.sync.dma_start(out=dst_rows3, in_=t16[:])

            def ag_chunk(li, c):
                src = hloc[li % 2][c]
                dstt = tab[(li + 1) % 2]
                nc.gpsimd.collective_compute(
                    "AllGather", mybir.AluOpType.bypass,
                    replica_groups=RG,
                    ins=[src[:, :]],
                    outs=[dstt[c * CHSH * NCORES:(c + 1) * CHSH * NCORES, :]],
                )

            # ---- encoder: h0 = x @ encW + encb -> hloc[1] ----
            with nc.named_scope("encoder"):
                for b in range(NBLK):
                    xt = mp.tile([8, 512], F32, tag="xt")
                    nc.sync.dma_start(out=xt[:],
                                      in_=xT_d[:, b * 512:(b + 1) * 512])
                    psum_y = ppy.tile([128, 512], F32, tag="py")
                    nc.tensor.matmul(out=psum_y[:], lhsT=encW_sb[:],
                                     rhs=xt[:], start=True, stop=True)
                    y_sb = mp.tile([128, 512], F32, tag="y_sb")
                    nc.vector.tensor_scalar_add(out=y_sb[:], in0=psum_y[:],
                                                scalar1=encb_sb[:, :])
                    transpose_store(y_sb, None, rows3c(hloc[1], b), False)
                    if (b + 1) % BPC == 0:
                        ag_chunk(-1, b // BPC)

            # ---- 6 GCN layers ----
            import os as _os2
            _nlayers = int(_os2.environ.get("KERNEL_NLAYERS", str(LAYERS)))
            _nbrlim = int(_os2.environ.get("KERNEL_NBR_LIM", str(NBR)))
            for li in range(_nlayers):
                t_cur = tab[li % 2]
                h_prev = hloc[(li + 1) % 2]
                h_next = hloc[li % 2]
                last = (li == LAYERS - 1)
                with nc.named_scope(f"layer{li}"):
                    for r in range(_nbrlim):
                        g0 = int(gofs[r])
                        G_r = int(gofs[r + 1]) - g0
                        Msup = gp.tile([128, GMAX, 128], F16, tag="Msup")
                        _cwcap = int(_os2.environ.get("KERNEL_CW_CAP", "6"))
                        if not _os2.environ.get("KERNEL_NO_GATHER"):
                            for (w, c0, cw) in calls[r]:
                                for cs in range(0, cw, _cwcap):
                                    cn = min(_cwcap, cw - cs)
                                    cc0 = c0 + cs
                                    nc.gpsimd.dma_gather(
                                        out_ap=Msup[:, cc0 - g0:
                                                    cc0 - g0 + cn, :],
                                        in_ap=t_cur[w * W0:(w + 1) * W0, :],
                                        idxs_ap=idx16_sb[:, cc0 * 8:
                                                         (cc0 + cn) * 8],
                                        num_idxs=cn * 128,
                                        num_idxs_reg=cn * 128,
                                        elem_size=128,
                                        single_packet=not _os2.environ.get(
                                            "KERNEL_MULTI_PACKET"))
                        else:
                            nc.vector.memset(Msup[:, :, :], 0.25)
                        nb = norm_sb[:, g0:g0 + G_r][:, :, None] \
                            .to_broadcast([128, G_r, 128])
                        nc.vector.tensor_tensor(
                            out=Msup[:, :G_r, :], in0=Msup[:, :G_r, :],
                            in1=nb, op=mybir.AluOpType.mult)
                        if _os2.environ.get("KERNEL_GATHER_ONLY"):
                            continue
                        for b in range(r * BRSZ, (r + 1) * BRSZ):
                            psum_s = pps.tile([128, 512], F32, tag="ps")
                            rsd = mp.tile([128, 4, 128], F16, tag="rsd")
                            nc.sync.dma_start(out=rsd[:],
                                              in_=rows3c(h_prev, b))
                            first = True
                            for (cc, w) in blockcols[b]:
                                S = sp.tile([128, 512], F16, tag="S")
                                nc.vector.tensor_scalar(
                                    out=S[:], in0=iota512[:],
                                    scalar1=dstloc_sb[:, cc:cc + 1],
                                    scalar2=None,
                                    op0=mybir.AluOpType.is_equal)
                                nc.tensor.matmul(
                                    out=psum_s[:], lhsT=Msup[:, cc - g0, :],
                                    rhs=S[:], start=first, stop=False,
                                    skip_group_check=True)
                                first = False
                            for q in range(4):
                                Ssl = sp.tile([128, 128], F16, tag="Ssl")
                                nc.scalar.activation(
                                    out=Ssl[:], in_=ident16[:],
                                    func=mybir.ActivationFunctionType.Copy,
                                    scale=snorm_sb[:,
                                                   b * 4 + q:b * 4 + q + 1])
                                nc.tensor.matmul(
                                    out=psum_s[:, q * 128:(q + 1) * 128],
                                    lhsT=rsd[:, q, :], rhs=Ssl[:],
                                    start=first, stop=True,
                                    skip_group_check=True)
                            first = False
                            s_sb = mp.tile([128, 512], F16, tag="s_sb")
                            nc.scalar.activation(
                                out=s_sb[:], in_=psum_s[:],
                                func=mybir.ActivationFunctionType.Copy)
                            psum_y = ppy.tile([128, 512], F32, tag="py")
                            nc.tensor.matmul(
                                out=psum_y[:],
                                lhsT=convW_sb[:, li * 128:(li + 1) * 128],
                                rhs=s_sb[:], start=True, stop=True)
                            y_sb = mp.tile([128, 512], F32, tag="y_sb")
                            nc.scalar.activation(
                                out=y_sb[:], in_=psum_y[:],
                                func=mybir.ActivationFunctionType.Relu,
                                bias=bnshift_sb[:, li:li + 1],
                                scale=bnscale_sb[:, li:li + 1])
                            transpose_store(
                                y_sb, rsd,
                                rows3(hloc6, b) if last else rows3c(h_next, b),
                                True)
                            if (not last and (b + 1) % BPC == 0
                                    and not _os2.environ.get("KERNEL_NO_AG")):
                                ag_chunk(li, b // BPC)

            # ---- final edge MLP ----
            import os as _os
            _skip_final = bool(int(_os.environ.get("SKIP_FINAL", "0")))
            h6 = hloc6
            fwin = [h6[0:FW0, :], h6[FW0:NSH, :]]
            with nc.named_scope("final"):
                for (uw, vw, gr0, ngr) in ([] if _skip_final else fcalls):
                    ehu = fgp.tile([128, 16, 128], F16, tag="ehu")
                    ehv = fgp.tile([128, 16, 128], F16, tag="ehv")
                    for cs in range(0, ngr, 6):
                        cn = min(6, ngr - cs)
                        nc.gpsimd.dma_gather(
                            out_ap=ehu[:, cs:cs + cn, :], in_ap=fwin[uw],
                            idxs_ap=fu16_sb[:, (gr0 + cs) * 8:
                                            (gr0 + cs + cn) * 8],
                            num_idxs=cn * 128, num_idxs_reg=cn * 128,
                            elem_size=128)
                        nc.gpsimd.dma_gather(
                            out_ap=ehv[:, cs:cs + cn, :], in_ap=fwin[vw],
                            idxs_ap=fv16_sb[:, (gr0 + cs) * 8:
                                            (gr0 + cs + cn) * 8],
                            num_idxs=cn * 128, num_idxs_reg=cn * 128,
                            elem_size=128)
                    hid16 = fgp.tile([128, 16, 128], F16, tag="hid16")
                    for tt in range(ngr):
                        psum_e = ppt.tile([128, 2, 128], F16, tag="pt16")
                        nc.tensor.matmul(out=psum_e[:, 0, :],
                                         lhsT=ehu[:, tt, :],
                                         rhs=ident16[:], is_transpose=True,
                                         start=True, stop=True,
                                         skip_group_check=True)
                        nc.tensor.matmul(out=psum_e[:, 1, :],
                                         lhsT=ehv[:, tt, :],
                                         rhs=ident16[:], is_transpose=True,
                                         start=True, stop=True,
                                         skip_group_check=True)
                        ehT = mp.tile([128, 2, 128], F16, tag="ehT")
                        nc.vector.tensor_copy(out=ehT[:], in_=psum_e[:])
                        psum_h = pps.tile([128, 128], F32, tag="ps")
                        nc.tensor.matmul(out=psum_h[:], lhsT=w1u_sb[:],
                                         rhs=ehT[:, 0, :], start=True,
                                         stop=False)
                        nc.tensor.matmul(out=psum_h[:], lhsT=w1v_sb[:],
                                         rhs=ehT[:, 1, :], start=False,
                                         stop=True)
                        nc.scalar.activation(
                            out=hid16[:, tt, :], in_=psum_h[:],
                            func=mybir.ActivationFunctionType.Relu,
                            bias=b1_sb[:, :], scale=1.0)
                    o_sb = fgp.tile([1, 2048], F32, tag="o_sb")
                    for half in range(4):
                        if half * 4 >= ngr:
                            break
                        nh = min(4, ngr - half * 4)
                        psum_o = ppy.tile([1, 512], F32, tag="py")
                        nc.tensor.matmul(
                            out=psum_o[:, :nh * 128], lhsT=w2_sb[:],
                            rhs=hid16[:, 4 * half:4 * half + nh, :].rearrange(
                                "p t f -> p (t f)"),
                            start=True, stop=True)
                        nc.vector.tensor_scalar_add(
                            out=o_sb[:, 512 * half:512 * half + nh * 128],
                            in0=psum_o[:, :nh * 128], scalar1=b2)
                    nc.sync.dma_start(
                        out=out_d[0:1, gr0 * 128:(gr0 + ngr) * 128],
                        in_=o_sb[:, :ngr * 128])

    nc.finalize()
    return nc


def kernel(**inputs):
    x = np.asarray(inputs["x"], np.float32)
    edge_index = np.asarray(inputs["edge_index"])
    assert x.shape == (N, 8)
    struct, data = _prep(x, edge_index)
    consts, b2 = _consts(
        np.asarray(inputs["enc_W"], np.float32),
        np.asarray(inputs["enc_b"], np.float32),
        np.asarray(inputs["conv_W"], np.float32),
        np.asarray(inputs["conv_b"], np.float32),
        np.asarray(inputs["bn_gamma"], np.float32),
        np.asarray(inputs["bn_beta"], np.float32),
        np.asarray(inputs["bn_mean"], np.float32),
        np.asarray(inputs["bn_var"], np.float32),
        np.asarray(inputs["mlp_W1"], np.float32),
        np.asarray(inputs["mlp_b1"], np.float32),
        np.asarray(inputs["mlp_W2"], np.float32),
        np.asarray(inputs["mlp_b2"], np.float32))

    key = "nc"
    if key not in _CACHE:
        _CACHE[key] = _build(struct, b2)
    nc = _CACHE[key]

    in_maps = []
    for k in range(NCORES):
        in_maps.append({
            "xT": data["xT"][k], "idx16": data["idx16"][k],
            "dstloc": data["dstloc"][k], "normv": data["normv"][k],
            "snorm": data["snorm"][k],
            "fu16": data["fu16"], "fv16": data["fv16"],
            "encW": consts["encW"], "encb": consts["encb"],
            "convW": consts["convW"], "bnscale": consts["bnscale"],
            "bnshift": consts["bnshift"], "w1u": consts["w1u"],
            "w1v": consts["w1v"], "w2": consts["w2"], "b1": consts["b1"],
        })

    trace = bool(int(__import__("os").environ.get("KERNEL_TRACE", "0")))
    res = run_bass_kernel_spmd(nc, in_maps, core_ids=list(range(NCORES)),
                               trace=trace)
    kernel.last_result = res
    edge_ids = struct["edge_ids"]
    valid = edge_ids >= 0
    out = np.empty((NCORES, NOUT), np.float32)
    for k in range(NCORES):
        r = np.asarray(res.results[k]["outd"], np.float32).reshape(-1)
        out[k][edge_ids[valid]] = r[valid]
    return out.reshape(NCORES * NOUT, 1)


# revision 42
# speedup vs baseline: 1.0990x; 1.0745x over previous
"""Trainium2 Bass kernel for nn_DeepEdgeCongestionGNN (6-layer GCN + edge MLP).

Strategy (8 NeuronCores, SPMD):
  - Nodes sharded by graph (2048 graphs = 61440 nodes per core), natural
    order. Per layer the full fp16 node table (chunk-interleaved rows) is
    AllGather-replicated; each core gathers the rows its edges need with
    dma_gather (int16 in-window indices; the 491520-row table splits into
    15 windows of 32768 rows).
  - Entries are sorted by (window, dst 512-node block) and padded to
    128-entry groups that are window-pure and block-pure. One dma_gather
    per (window, 8-block range) fetches ~1024 rows; a 512-wide one-hot
    matmul scatters each group into the block's PSUM bank (feat x node),
    self-loops enter via identity matmuls on the residual tile.
  - y^T = W^T s^T, fused BN+ReLU on ACT, transpose back node-major with
    the residual added on the PE, store shard, AllGather chunks.
  - Final edge MLP gathers h6[u],h6[v] via dma_gather (edges sorted by
    (u-window, v-window) class); host un-permutes the padded output.
  - The bass program is built per input (structure depends on edge counts)
    but is identical on all cores: group counts take the max over cores.
"""
import sys
import types

import numpy as np

sys.path.insert(0, "/opt/trn_rl_repo")

# --- shim antenv.axon_hooks (absent in this image) so trace=True works ---
import antenv
if "antenv.axon_hooks" not in sys.modules:
    _hookmod = types.ModuleType("antenv.axon_hooks")
    _hookmod._hook = None
    def _set(h): _hookmod._hook = h
    def _get(): return _hookmod._hook
    _hookmod.set_axon_ntff_profile_hook = _set
    _hookmod.get_axon_ntff_profile_hook = _get
    sys.modules["antenv.axon_hooks"] = _hookmod
    antenv.axon_hooks = _hookmod
    try:
        from trn_agent_boot.trn_boot import _ntff_profile_via_ctypes
        _hookmod._hook = _ntff_profile_via_ctypes("/opt/axon/libaxon_pjrt.so")
    except Exception:
        pass

import concourse.bass as bass
import concourse.bacc as bacc
import concourse.mybir as mybir
import concourse.tile as tile
from concourse.bass_utils import run_bass_kernel_spmd

F16 = mybir.dt.float16
F32 = mybir.dt.float32
I32 = mybir.dt.int32
I16 = mybir.dt.int16

NCORES = 8
NPG = 30                    # nodes per graph
G = 16384                   # graphs
N = G * NPG                 # 491520 nodes
GPC = G // NCORES           # graphs per core
NSH = GPC * NPG             # 61440 nodes per core
NBLK = NSH // 512           # 120 psum blocks of 512 nodes
LAYERS = 6
HID = 128
SPLIT = 4                   # AllGather chunks per layer
CHSH = NSH // SPLIT         # shard rows per AG chunk
BPC = NBLK // SPLIT         # blocks per AG chunk
W0 = 32768                  # dma_gather window rows (int16 idx range)
NWIN = N // W0              # 15 table windows
BRSZ = 6                    # blocks per gather range
NBR = NBLK // BRSZ          # 20 ranges
NGB = 16                    # max groups per block (S supertile cols)
FW0 = 32768                 # final-stage window over NSH rows
NFW = 2                     # final windows (61440 = 32768 + 28672)
NOUT = GPC * 41             # 83968 output rows per core
BN_EPS = 1e-5

BRANCH_U = np.array([0,0,1,2,1,1,3,5,5,6,6,6,6,8,8,9,11,11,11,11,13,15,14,17,
                     18,9,9,21,14,21,22,23,24,24,27,26,26,28,26,7,5],
                    dtype=np.int64)
BRANCH_V = np.array([1,2,3,3,4,5,5,6,7,7,8,9,27,9,10,10,12,13,15,16,14,16,17,
                     18,19,19,20,20,22,21,23,23,24,26,26,29,28,29,27,27,8],
                    dtype=np.int64)

_CACHE = {}


def _table_row(g):
    """Global node id -> row in the chunk-interleaved AllGather table."""
    k = g // NSH
    l = g % NSH
    return (l // CHSH) * (CHSH * NCORES) + k * CHSH + (l % CHSH)


def _prep(x, edge_index):
    src = np.ascontiguousarray(edge_index[0]).astype(np.int64)
    dst = np.ascontiguousarray(edge_index[1]).astype(np.int64)

    indeg = np.bincount(dst, minlength=N).astype(np.int64)
    deg = (indeg + 1).astype(np.float64)
    dinv = (1.0 / np.sqrt(deg)).astype(np.float32)

    e_k = dst // NSH                       # consumer core
    e_l = dst % NSH
    e_blk = (e_l // 512).astype(np.int64)
    e_dloc = (e_l % 512).astype(np.int64)
    srow = _table_row(src)
    e_w = srow // W0
    e_r16 = (srow % W0).astype(np.int64)
    e_norm = (dinv[src] * dinv[dst]).astype(np.float32)

    # counts per (core, window, block)
    key_wb = e_w * NBLK + e_blk
    cnt = np.zeros((NCORES, NWIN * NBLK), np.int64)
    for k in range(NCORES):
        m = e_k == k
        cnt[k] = np.bincount(key_wb[m], minlength=NWIN * NBLK)
    g_wb = np.ceil(cnt.max(axis=0) / 128).astype(np.int64)  # shared structure
    g_wb = g_wb.reshape(NWIN, NBLK)

    # column enumeration: range-major, then window, then block, then group
    col_of = {}
    blockcols = [[] for _ in range(NBLK)]   # per block: [(col, w)...]
    calls = [[] for _ in range(NBR)]        # per range: [(w, c0, cw)...]
    gofs = np.zeros(NBR + 1, np.int64)      # base col per range
    c = 0
    for r in range(NBR):
        gofs[r] = c
        for w in range(NWIN):
            c0 = c
            for b in range(r * BRSZ, (r + 1) * BRSZ):
                for gi in range(g_wb[w, b]):
                    col_of[(w, b, gi)] = c
                    blockcols[b].append((c, w))
                    c += 1
            if c > c0:
                calls[r].append((w, c0, c - c0))
    gofs[NBR] = c
    GT = c                                   # total groups
    GMAX = int((gofs[1:] - gofs[:-1]).max())

    # block-major column permutation for dstloc (one fused S build per block)
    ngb = np.array([len(blockcols[b]) for b in range(NBLK)], np.int64)
    assert ngb.max() <= NGB, ngb.max()
    bofs = np.zeros(NBLK + 1, np.int64)
    bofs[1:] = np.cumsum(ngb)
    invperm = np.empty(GT, np.int64)        # block-major pos -> group col
    p = 0
    for b in range(NBLK):
        for (cc, w) in blockcols[b]:
            invperm[p] = cc
            p += 1

    # per-core gather data
    idx16 = np.zeros((NCORES, 128, GT * 8), np.int16)
    dstloc = np.full((NCORES, 128, GT), 999.0, np.float32)
    normv = np.zeros((NCORES, 128, GT), np.float32)
    for k in range(NCORES):
        m = np.where(e_k == k)[0]
        order = m[np.lexsort((e_blk[m], e_w[m]))]
        ws = e_w[order]
        bs = e_blk[order]
        r16s = e_r16[order]
        dls = e_dloc[order]
        nms = e_norm[order]
        # within each (w,b) run, positions 0..cnt-1
        kw = ws * NBLK + bs
        chg = np.empty(kw.shape[0], np.bool_)
        chg[0] = True
        chg[1:] = kw[1:] != kw[:-1]
        starts = np.where(chg)[0]
        run_id = np.cumsum(chg) - 1
        pos = np.arange(kw.shape[0]) - starts[run_id]
        base_col = np.array([col_of[(w, b, 0)] for (w, b) in
                             zip(ws[starts], bs[starts])], np.int64)
        colv = base_col[run_id] + pos // 128
        j = pos % 128
        idx16[k][j % 16, colv * 8 + j // 16] = r16s
        dstloc[k][j, colv] = dls.astype(np.float32)
        normv[k][j, colv] = nms
        # pads keep idx 0 (valid row in window), dstloc 999 (no one-hot hit)
        idx16[k][16:, :] = np.tile(idx16[k][:16, :], (7, 1))
    dstloc = dstloc[:, :, invperm].copy()    # block-major columns

    # self-loop scale dinv^2, [128, NSLOT] natural order per core
    NSLOT = NSH // 128
    sn = (dinv * dinv).astype(np.float32)
    snorm = np.empty((NCORES, 128, NSLOT), np.float32)
    for k in range(NCORES):
        snorm[k] = sn[k * NSH:(k + 1) * NSH].reshape(NSLOT, 128).T

    # x transposed per core
    xT = np.empty((NCORES, 8, NSH), np.float32)
    for k in range(NCORES):
        xT[k] = x[k * NSH:(k + 1) * NSH].T

    # ---- final stage: identical structure on every core ----
    goff = (np.arange(GPC, dtype=np.int64) * NPG)[:, None]
    u = (goff + BRANCH_U[None, :]).reshape(-1)   # [NOUT] local rows
    v = (goff + BRANCH_V[None, :]).reshape(-1)
    cls = (u // FW0) * 2 + (v // FW0)
    order_f = np.argsort(cls, kind="stable")
    ccnt = np.bincount(cls, minlength=4)
    cpad = (np.ceil(ccnt / 128) * 128).astype(np.int64)
    NOUTP = int(cpad.sum())
    fu = np.zeros(NOUTP, np.int64)
    fv = np.zeros(NOUTP, np.int64)
    edge_ids = np.full(NOUTP, -1, np.int64)
    fcalls = []        # (uwin, vwin, group0, ngroups) per chunk call
    p0 = 0
    o0 = 0
    for cc in range(4):
        n = int(ccnt[cc])
        sel = order_f[o0:o0 + n]
        fu[p0:p0 + n] = u[sel]
        fv[p0:p0 + n] = v[sel]
        fu[p0 + n:p0 + int(cpad[cc])] = (cc // 2) * FW0
        fv[p0 + n:p0 + int(cpad[cc])] = (cc % 2) * FW0
        edge_ids[p0:p0 + n] = sel
        ngr = int(cpad[cc]) // 128
        g0 = p0 // 128
        for s in range(0, ngr, 16):
            fcalls.append((cc // 2, cc % 2, g0 + s, min(16, ngr - s)))
        p0 += int(cpad[cc])
        o0 += n
    fu16 = np.zeros((128, NOUTP // 16), np.int16)
    fv16 = np.zeros((128, NOUTP // 16), np.int16)
    j = np.arange(NOUTP)
    fu16[j % 16, j // 16] = (fu % FW0).astype(np.int16)
    fv16[j % 16, j // 16] = (fv % FW0).astype(np.int16)
    fu16[16:, :] = np.tile(fu16[:16, :], (7, 1))
    fv16[16:, :] = np.tile(fv16[:16, :], (7, 1))

    struct = dict(g_wb=g_wb, blockcols=blockcols, calls=calls, gofs=gofs,
                  GT=GT, GMAX=GMAX, fcalls=fcalls, NOUTP=NOUTP,
                  edge_ids=edge_ids, bofs=bofs, ngb=ngb)
    data = dict(idx16=idx16, dstloc=dstloc, normv=normv, snorm=snorm, xT=xT,
                fu16=fu16, fv16=fv16)
    return struct, data


def _consts(enc_W, enc_b, conv_W, conv_b, bn_gamma, bn_beta, bn_mean, bn_var,
            mlp_W1, mlp_b1, mlp_W2, mlp_b2):
    bnscale = (bn_gamma / np.sqrt(bn_var + BN_EPS)).astype(np.float32)
    bnshift = ((conv_b - bn_mean) * bnscale + bn_beta).astype(np.float32)
    consts = dict(
        encW=enc_W.astype(np.float32),                       # [8,128]
        encb=enc_b.reshape(128, 1).astype(np.float32),
        convW=np.concatenate([conv_W[i] for i in range(LAYERS)], axis=1
                             ).astype(np.float16),           # [128, 768]
        bnscale=bnscale.T.copy(),                            # [128, 6]
        bnshift=bnshift.T.copy(),
        w1u=mlp_W1[:128].astype(np.float16),
        w1v=mlp_W1[128:].astype(np.float16),
        w2=mlp_W2.astype(np.float16),                        # [128,1]
        b1=mlp_b1.reshape(128, 1).astype(np.float32),
    )
    b2 = float(np.asarray(mlp_b2).reshape(-1)[0])
    return consts, b2


def _build(struct, b2):
    GT = struct["GT"]
    GMAX = struct["GMAX"]
    gofs = struct["gofs"]
    calls = struct["calls"]
    blockcols = struct["blockcols"]
    fcalls = struct["fcalls"]
    NOUTP = struct["NOUTP"]
    bofs = struct["bofs"]
    ngb = struct["ngb"]
    NSLOT = NSH // 128
    NFCH = NOUTP // 2048 if NOUTP % 2048 == 0 else None

    nc = bacc.Bacc("TRN2", target_bir_lowering=False, debug=False,
                   num_devices=NCORES)

    xT_d = nc.dram_tensor("xT", [8, NSH], F32, kind="ExternalInput")
    idx16_d = nc.dram_tensor("idx16", [128, GT * 8], I16,
                             kind="ExternalInput")
    dstloc_d = nc.dram_tensor("dstloc", [128, GT], F32, kind="ExternalInput")
    norm_d = nc.dram_tensor("normv", [128, GT], F32, kind="ExternalInput")
    fu16_d = nc.dram_tensor("fu16", [128, NOUTP // 16], I16,
                            kind="ExternalInput")
    fv16_d = nc.dram_tensor("fv16", [128, NOUTP // 16], I16,
                            kind="ExternalInput")
    encW_d = nc.dram_tensor("encW", [8, 128], F32, kind="ExternalInput")
    encb_d = nc.dram_tensor("encb", [128, 1], F32, kind="ExternalInput")
    convW_d = nc.dram_tensor("convW", [128, LAYERS * 128], F16,
                             kind="ExternalInput")
    bnscale_d = nc.dram_tensor("bnscale", [128, LAYERS], F32,
                               kind="ExternalInput")
    bnshift_d = nc.dram_tensor("bnshift", [128, LAYERS], F32,
                               kind="ExternalInput")
    w1u_d = nc.dram_tensor("w1u", [128, 128], F16, kind="ExternalInput")
    w1v_d = nc.dram_tensor("w1v", [128, 128], F16, kind="ExternalInput")
    w2_d = nc.dram_tensor("w2", [128, 1], F16, kind="ExternalInput")
    b1_d = nc.dram_tensor("b1", [128, 1], F32, kind="ExternalInput")
    snorm_d = nc.dram_tensor("snorm", [128, NSLOT], F32, kind="ExternalInput")

    out_d = nc.dram_tensor("outd", [1, NOUTP], F32, kind="ExternalOutput")

    hloc = [[nc.dram_tensor(f"hloc{j}_{c}", [CHSH, 128], F16,
                            kind="Internal") for c in range(SPLIT)]
            for j in range(2)]
    hloc6 = nc.dram_tensor("hloc6", [NSH, 128], F16, kind="Internal")
    tab = [nc.dram_tensor(f"tab{j}", [N, 128], F16, kind="Internal",
                          addr_space="Shared") for j in range(2)]
    RG = [list(range(NCORES))]

    def rows3(t, b):
        return t[b * 512:(b + 1) * 512, :].rearrange("(j p) f -> p j f", p=128)

    def rows3c(hl, b):
        return rows3(hl[b // BPC], b % BPC)

    with tile.TileContext(nc) as tc:
        with (
            tc.tile_pool(name="const", bufs=1) as cpool,
            tc.tile_pool(name="gat",
                         bufs=int(__import__("os").environ.get(
                             "KERNEL_GP_BUFS", "2"))) as gp,
            tc.tile_pool(name="fgat", bufs=2) as fgp,
            tc.tile_pool(name="sone", bufs=4) as sp,
            tc.tile_pool(name="sblk", bufs=2) as sb_pool,
            tc.tile_pool(name="mid", bufs=2) as mp,
            tc.tile_pool(name="pps", bufs=2, space="PSUM") as pps,
            tc.tile_pool(name="ppy", bufs=2, space="PSUM") as ppy,
            tc.tile_pool(name="ppt", bufs=2, space="PSUM") as ppt,
        ):
            # ---- constants ----
            iota_i = cpool.tile([128, 128], I16, tag="iotai")
            nc.gpsimd.iota(iota_i[:], pattern=[[1, 128]], base=0,
                           channel_multiplier=0)
            prow_i = cpool.tile([128, 128], I16, tag="prowi")
            nc.gpsimd.iota(prow_i[:], pattern=[[0, 128]], base=0,
                           channel_multiplier=1)
            ident32 = cpool.tile([128, 128], F32, tag="id32")
            nc.vector.tensor_tensor(out=ident32[:], in0=prow_i[:],
                                    in1=iota_i[:], op=mybir.AluOpType.is_equal)
            ident16 = cpool.tile([128, 128], F16, tag="id16")
            nc.vector.tensor_copy(out=ident16[:], in_=ident32[:])
            iota5_i = cpool.tile([128, 512], I16, tag="iota5i")
            nc.gpsimd.iota(iota5_i[:], pattern=[[1, 512]], base=0,
                           channel_multiplier=0)
            iota512 = cpool.tile([128, 512], F16, tag="iota512")
            nc.vector.tensor_copy(out=iota512[:], in_=iota5_i[:])

            idx16_sb = cpool.tile([128, GT * 8], I16, tag="idx16")
            nc.sync.dma_start(out=idx16_sb[:], in_=idx16_d[:, :])
            dstloc_sb = cpool.tile([128, GT], F32, tag="dstloc")
            nc.sync.dma_start(out=dstloc_sb[:], in_=dstloc_d[:, :])
            norm_sb = cpool.tile([128, GT], F32, tag="norm")
            nc.sync.dma_start(out=norm_sb[:], in_=norm_d[:, :])

            encW_sb = cpool.tile([8, 128], F32, tag="encW")
            nc.sync.dma_start(out=encW_sb[:], in_=encW_d[:, :])
            encb_sb = cpool.tile([128, 1], F32, tag="encb")
            nc.sync.dma_start(out=encb_sb[:], in_=encb_d[:, :])
            convW_sb = cpool.tile([128, LAYERS * 128], F16, tag="convW")
            nc.sync.dma_start(out=convW_sb[:], in_=convW_d[:, :])
            bnscale_sb = cpool.tile([128, LAYERS], F32, tag="bns")
            nc.sync.dma_start(out=bnscale_sb[:], in_=bnscale_d[:, :])
            bnshift_sb = cpool.tile([128, LAYERS], F32, tag="bnsh")
            nc.sync.dma_start(out=bnshift_sb[:], in_=bnshift_d[:, :])
            w1u_sb = cpool.tile([128, 128], F16, tag="w1u")
            nc.sync.dma_start(out=w1u_sb[:], in_=w1u_d[:, :])
            w1v_sb = cpool.tile([128, 128], F16, tag="w1v")
            nc.sync.dma_start(out=w1v_sb[:], in_=w1v_d[:, :])
            w2_sb = cpool.tile([128, 1], F16, tag="w2")
            nc.sync.dma_start(out=w2_sb[:], in_=w2_d[:, :])
            b1_sb = cpool.tile([128, 1], F32, tag="b1")
            nc.sync.dma_start(out=b1_sb[:], in_=b1_d[:, :])
            snorm_sb = cpool.tile([128, NSLOT], F32, tag="snorm")
            nc.sync.dma_start(out=snorm_sb[:], in_=snorm_d[:, :])

            def transpose_store(y_sb, rsd, dst_rows3, with_res):
                psum_t = ppt.tile([128, 4, 128], F32, tag="pt")
                for j in range(4):
                    nc.tensor.matmul(
                        out=psum_t[:, j, :],
                        lhsT=y_sb[:, j * 128:(j + 1) * 128],
                        rhs=ident32[:], is_transpose=True,
                        start=True, stop=(not with_res),
                        skip_group_check=True)
                    if with_res:
                        nc.tensor.matmul(
                            out=psum_t[:, j, :], lhsT=ident16[:],
                            rhs=rsd[:, j, :], start=False, stop=True,
                            skip_group_check=True)
                t16 = mp.tile([128, 4, 128], F16, tag="t16")
                nc.scalar.activation(out=t16[:], in_=psum_t[:],
                                     func=mybir.ActivationFunctionType.Copy)
                nc.sync.dma_start(out=dst_rows3, in_=t16[:])

            def ag_chunk(li, c):
                src = hloc[li % 2][c]
                dstt = tab[(li + 1) % 2]
                nc.gpsimd.collective_compute(
                    "AllGather", mybir.AluOpType.bypass,
                    replica_groups=RG,
                    ins=[src[:, :]],
                    outs=[dstt[c * CHSH * NCORES:(c + 1) * CHSH * NCORES, :]],
                )

            # ---- encoder: h0 = x @ encW + encb -> hloc[1] ----
            with nc.named_scope("encoder"):
                for b in range(NBLK):
                    xt = mp.tile([8, 512], F32, tag="xt")
                    nc.sync.dma_start(out=xt[:],
                                      in_=xT_d[:, b * 512:(b + 1) * 512])
                    psum_y = ppy.tile([128, 512], F32, tag="py")
                    nc.tensor.matmul(out=psum_y[:], lhsT=encW_sb[:],
                                     rhs=xt[:], start=True, stop=True)
                    y_sb = mp.tile([128, 512], F32, tag="y_sb")
                    nc.vector.tensor_scalar_add(out=y_sb[:], in0=psum_y[:],
                                                scalar1=encb_sb[:, :])
                    transpose_store(y_sb, None, rows3c(hloc[1], b), False)
                    if (b + 1) % BPC == 0:
                        ag_chunk(-1, b // BPC)

            # ---- 6 GCN layers ----
            import os as _os2
            _nlayers = int(_os2.environ.get("KERNEL_NLAYERS", str(LAYERS)))
            _nbrlim = int(_os2.environ.get("KERNEL_NBR_LIM", str(NBR)))
            for li in range(_nlayers):
                t_cur = tab[li % 2]
                h_prev = hloc[(li + 1) % 2]
                h_next = hloc[li % 2]
                last = (li == LAYERS - 1)
                with nc.named_scope(f"layer{li}"):
                    for r in range(_nbrlim):
                        g0 = int(gofs[r])
                        G_r = int(gofs[r + 1]) - g0
                        Msup = gp.tile([128, GMAX, 128], F16, tag="Msup")
                        _cwcap = int(_os2.environ.get("KERNEL_CW_CAP", "6"))
                        if not _os2.environ.get("KERNEL_NO_GATHER"):
                            for (w, c0, cw) in calls[r]:
                                for cs in range(0, cw, _cwcap):
                                    cn = min(_cwcap, cw - cs)
                                    cc0 = c0 + cs
                                    nc.gpsimd.dma_gather(
                                        out_ap=Msup[:, cc0 - g0:
                                                    cc0 - g0 + cn, :],
                                        in_ap=t_cur[w * W0:(w + 1) * W0, :],
                                        idxs_ap=idx16_sb[:, cc0 * 8:
                                                         (cc0 + cn) * 8],
                                        num_idxs=cn * 128,
                                        num_idxs_reg=cn * 128,
                                        elem_size=128,
                                        single_packet=not _os2.environ.get(
                                            "KERNEL_MULTI_PACKET"))
                        else:
                            nc.vector.memset(Msup[:, :, :], 0.25)
                        nb = norm_sb[:, g0:g0 + G_r][:, :, None] \
                            .to_broadcast([128, G_r, 128])
                        nc.vector.tensor_tensor(
                            out=Msup[:, :G_r, :], in0=Msup[:, :G_r, :],
                            in1=nb, op=mybir.AluOpType.mult)
                        if _os2.environ.get("KERNEL_GATHER_ONLY"):
                            continue
                        for b in range(r * BRSZ, (r + 1) * BRSZ):
                            psum_s = pps.tile([128, 512], F32, tag="ps")
                            rsd = mp.tile([128, 4, 128], F16, tag="rsd")
                            nc.sync.dma_start(out=rsd[:],
                                              in_=rows3c(h_prev, b))
                            nb_b = int(ngb[b])
                            b0 = int(bofs[b])
                            S_blk = sb_pool.tile([128, NGB, 512], F16,
                                                 tag="Sb")
                            nc.vector.tensor_tensor(
                                out=S_blk[:, :nb_b, :],
                                in0=dstloc_sb[:, b0:b0 + nb_b][:, :, None]
                                .to_broadcast([128, nb_b, 512]),
                                in1=iota512[:][:, None, :]
                                .to_broadcast([128, nb_b, 512]),
                                op=mybir.AluOpType.is_equal)
                            first = True
                            for gi, (cc, w) in enumerate(blockcols[b]):
                                nc.tensor.matmul(
                                    out=psum_s[:], lhsT=Msup[:, cc - g0, :],
                                    rhs=S_blk[:, gi, :], start=first,
                                    stop=False, skip_group_check=True)
                                first = False
                            for q in range(4):
                                Ssl = sp.tile([128, 128], F16, tag="Ssl")
                                nc.scalar.activation(
                                    out=Ssl[:], in_=ident16[:],
                                    func=mybir.ActivationFunctionType.Copy,
                                    scale=snorm_sb[:,
                                                   b * 4 + q:b * 4 + q + 1])
                                nc.tensor.matmul(
                                    out=psum_s[:, q * 128:(q + 1) * 128],
                                    lhsT=rsd[:, q, :], rhs=Ssl[:],
                                    start=first, stop=True,
                                    skip_group_check=True)
                            first = False
                            s_sb = mp.tile([128, 512], F16, tag="s_sb")
                            nc.scalar.activation(
                                out=s_sb[:], in_=psum_s[:],
                                func=mybir.ActivationFunctionType.Copy)
                            psum_y = ppy.tile([128, 512], F32, tag="py")
                            nc.tensor.matmul(
                                out=psum_y[:],
                                lhsT=convW_sb[:, li * 128:(li + 1) * 128],
                                rhs=s_sb[:], start=True, stop=True)
                            y_sb = mp.tile([128, 512], F32, tag="y_sb")
                            nc.scalar.activation(
                                out=y_sb[:], in_=psum_y[:],
                                func=mybir.ActivationFunctionType.Relu,
                                bias=bnshift_sb[:, li:li + 1],
                                scale=bnscale_sb[:, li:li + 1])
                            transpose_store(
                                y_sb, rsd,
                                rows3(hloc6, b) if last else rows3c(h_next, b),
                                True)
                            if (not last and (b + 1) % BPC == 0
                                    and not _os2.environ.get("KERNEL_NO_AG")):
                                ag_chunk(li, b // BPC)

            # ---- final edge MLP ----
            import os as _os
            _skip_final = bool(int(_os.environ.get("SKIP_FINAL", "0")))
            h6 = hloc6
            fwin = [h6[0:FW0, :], h6[FW0:NSH, :]]
            with nc.named_scope("final"):
                for (uw, vw, gr0, ngr) in ([] if _skip_final else fcalls):
                    fub = mp.tile([128, 16 * 8], I16, tag="fub")
                    nc.sync.dma_start(
                        out=fub[:, :ngr * 8],
                        in_=fu16_d[:, gr0 * 8:(gr0 + ngr) * 8])
                    fvb = mp.tile([128, 16 * 8], I16, tag="fvb")
                    nc.sync.dma_start(
                        out=fvb[:, :ngr * 8],
                        in_=fv16_d[:, gr0 * 8:(gr0 + ngr) * 8])
                    ehu = fgp.tile([128, 16, 128], F16, tag="ehu")
                    ehv = fgp.tile([128, 16, 128], F16, tag="ehv")
                    for cs in range(0, ngr, 6):
                        cn = min(6, ngr - cs)
                        nc.gpsimd.dma_gather(
                            out_ap=ehu[:, cs:cs + cn, :], in_ap=fwin[uw],
                            idxs_ap=fub[:, cs * 8:(cs + cn) * 8],
                            num_idxs=cn * 128, num_idxs_reg=cn * 128,
                            elem_size=128)
                        nc.gpsimd.dma_gather(
                            out_ap=ehv[:, cs:cs + cn, :], in_ap=fwin[vw],
                            idxs_ap=fvb[:, cs * 8:(cs + cn) * 8],
                            num_idxs=cn * 128, num_idxs_reg=cn * 128,
                            elem_size=128)
                    hid16 = fgp.tile([128, 16, 128], F16, tag="hid16")
                    for tt in range(ngr):
                        psum_e = ppt.tile([128, 2, 128], F16, tag="pt16")
                        nc.tensor.matmul(out=psum_e[:, 0, :],
                                         lhsT=ehu[:, tt, :],
                                         rhs=ident16[:], is_transpose=True,
                                         start=True, stop=True,
                                         skip_group_check=True)
                        nc.tensor.matmul(out=psum_e[:, 1, :],
                                         lhsT=ehv[:, tt, :],
                                         rhs=ident16[:], is_transpose=True,
                                         start=True, stop=True,
                                         skip_group_check=True)
                        ehT = mp.tile([128, 2, 128], F16, tag="ehT")
                        nc.vector.tensor_copy(out=ehT[:], in_=psum_e[:])
                        psum_h = pps.tile([128, 128], F32, tag="ps")
                        nc.tensor.matmul(out=psum_h[:], lhsT=w1u_sb[:],
                                         rhs=ehT[:, 0, :], start=True,
                                         stop=False)
                        nc.tensor.matmul(out=psum_h[:], lhsT=w1v_sb[:],
                                         rhs=ehT[:, 1, :], start=False,
                                         stop=True)
                        nc.scalar.activation(
                            out=hid16[:, tt, :], in_=psum_h[:],
                            func=mybir.ActivationFunctionType.Relu,
                            bias=b1_sb[:, :], scale=1.0)
                    o_sb = fgp.tile([1, 2048], F32, tag="o_sb")
                    for half in range(4):
                        if half * 4 >= ngr:
                            break
                        nh = min(4, ngr - half * 4)
                        psum_o = ppy.tile([1, 512], F32, tag="py")
                        nc.tensor.matmul(
                            out=psum_o[:, :nh * 128], lhsT=w2_sb[:],
                            rhs=hid16[:, 4 * half:4 * half + nh, :].rearrange(
                                "p t f -> p (t f)"),
                            start=True, stop=True)
                        nc.vector.tensor_scalar_add(
                            out=o_sb[:, 512 * half:512 * half + nh * 128],
                            in0=psum_o[:, :nh * 128], scalar1=b2)
                    nc.sync.dma_start(
                        out=out_d[0:1, gr0 * 128:(gr0 + ngr) * 128],
                        in_=o_sb[:, :ngr * 128])

    nc.finalize()
    return nc


def kernel(**inputs):
    x = np.asarray(inputs["x"], np.float32)
    edge_index = np.asarray(inputs["edge_index"])
    assert x.shape == (N, 8)
    struct, data = _prep(x, edge_index)
    consts, b2 = _consts(
        np.asarray(inputs["enc_W"], np.float32),
        np.asarray(inputs["enc_b"], np.float32),
        np.asarray(inputs["conv_W"], np.float32),
        np.asarray(inputs["conv_b"], np.float32),
        np.asarray(inputs["bn_gamma"], np.float32),
        np.asarray(inputs["bn_beta"], np.float32),
        np.asarray(inputs["bn_mean"], np.float32),
        np.asarray(inputs["bn_var"], np.float32),
        np.asarray(inputs["mlp_W1"], np.float32),
        np.asarray(inputs["mlp_b1"], np.float32),
        np.asarray(inputs["mlp_W2"], np.float32),
        np.asarray(inputs["mlp_b2"], np.float32))

    key = "nc"
    if key not in _CACHE:
        _CACHE[key] = _build(struct, b2)
    nc = _CACHE[key]

    in_maps = []
    for k in range(NCORES):
        in_maps.append({
            "xT": data["xT"][k], "idx16": data["idx16"][k],
            "dstloc": data["dstloc"][k], "normv": data["normv"][k],
            "snorm": data["snorm"][k],
            "fu16": data["fu16"], "fv16": data["fv16"],
            "encW": consts["encW"], "encb": consts["encb"],
            "convW": consts["convW"], "bnscale": consts["bnscale"],
            "bnshift": consts["bnshift"], "w1u": consts["w1u"],
            "w1v": consts["w1v"], "w2": consts["w2"], "b1": consts["b1"],
        })

    trace = bool(int(__import__("os").environ.get("KERNEL_TRACE", "0")))
    res = run_bass_kernel_spmd(nc, in_maps, core_ids=list(range(NCORES)),
                               trace=trace)
    kernel.last_result = res
    edge_ids = struct["edge_ids"]
    valid = edge_ids >= 0
    out = np.empty((NCORES, NOUT), np.float32)
    for k in range(NCORES):
        r = np.asarray(res.results[k]["outd"], np.float32).reshape(-1)
        out[k][edge_ids[valid]] = r[valid]
    return out.reshape(NCORES * NOUT, 1)
